# revision 33
# baseline (speedup 1.0000x reference)
"""Trainium2 Bass kernel for 16-head self-attention (D=1024, S=2048, B=2)
with upper-triangular (j >= i) mask and scale 1/head_dim.

Sharding: batch*head-group parallel over 8 cores. Core c handles batch
c//4, heads [4*(c%4), 4*(c%4)+4). Each core computes Q/K/V projections for
its 256 output dims, attention for its 4 heads, and a partial output
projection (its 256 rows of wo). Host sums the 4 partials per batch.

On-chip layout is transposed end-to-end: QT/KT [dh, seq], scores S^T
[seq_k, seq_q] (stationary=K^T chunk, moving=Q^T), exp on ScalarE
PSUM->SBUF with the 1/64 scale folded in, PV as O'^T = V'^T E^T with a
ones-column appended to V so row 64 of O' is the softmax denominator,
then out^T = wo^T O^T. The host transposes back.

Perf notes (an 8-core run sits against a chip activity/power clamp -
the PE clock-gate holds ~1.2-1.4GHz effective through the attention
phase - so total switched work matters as much as overlap):
 - Q/K projections in fp8 (e4m3) DoubleRow matmuls: 256-deep contraction
   at 0.5 cyc/col - 4 instead of 8 matmuls per PSUM tile and half the
   streamed bytes. fp8 error on q/k (~4%) only perturbs the softmax
   scores by ~0.3% absolute (score scale is tiny) - measured output
   rel err stays ~5.5e-3. V/O paths stay bf16 (fp8 there costs ~3%
   output error - over the 2e-2 budget).
 - attention runs per-head in two q-halves so O' fits 2 PSUM banks,
   leaving 6 banks for a triple-buffered scores pipeline; PV pieces of
   chunk jc-1 are emitted between the scores groups of chunk jc so the
   PE never drains while ScalarE runs exp.
 - PV writes its diagonal block at exact width; start_tensor_calc
   zeroes the whole 2KB PSUM bank so no e-tail zero-fill is needed.
 - the rust scheduler splits every matmul into LDWEIGHTS+MATMUL;
   _dedup_ldweights removes ~190 reloads of an unchanged stationary
   (the walrus --enable-ldw-opt pass is incompatible with bass IR).
 - softmax 1/denom = exp(-ln d) on ScalarE (5e-5 rel err; DVE's
   reciprocal op costs ~6 cyc/element and 13us/head).
 - output partials stored bf16 (halves output HBM traffic); host sums
   the four per-batch partials and adds the output bias.
"""

import sys

sys.path.insert(0, "/opt/trn_rl_repo")

import numpy as np

import concourse.bass as bass
import concourse.mybir as mybir
from concourse import tile
from concourse.bass_utils import run_bass_kernel_spmd

# ---------------------------------------------------------------------------
# Workaround: this walrus build supports only 1 sync wait on the SP CTRL
# (drain) instruction; split the TileContext exit drain's waits across
# sequential drains (same-engine program order makes this equivalent).
_MAX_DRAIN_WAITS = 1


def _patched_drain_and_barrier(self, tick_clock, wait_clock):
    from bass_rust import ScopedClock

    nc = self.nc
    drain_inst = nc.sync.drain()
    wait_clock.add_sem_waits(
        drain_inst.ins, ScopedClock({None: tick_clock.global_clock})
    )
    si = drain_inst.ins.sync_info
    if si is not None and len(si.on_wait) > _MAX_DRAIN_WAITS:
        waits = list(si.on_wait)
        si.on_wait = waits[:_MAX_DRAIN_WAITS]
        rest = waits[_MAX_DRAIN_WAITS:]
        while rest:
            chunk, rest = rest[:_MAX_DRAIN_WAITS], rest[_MAX_DRAIN_WAITS:]
            extra = nc.sync.drain()
            esi = extra.ins.sync_info
            if esi is None:
                extra.ins.sync_info = mybir.SyncInfo(on_wait=chunk, on_update=[])
            else:
                esi.on_wait = chunk
    nc.all_engine_barrier()
    assert self.sems is not None
    popped = nc._tile_sem_poison_stack.pop()
    assert popped is self._sem_poison
    nc.clear_and_free_semaphores(list(self.sems.allocated().values()))
    nc.all_engine_barrier()


tile.TileContext._drain_and_barrier = _patched_drain_and_barrier


def _dedup_ldweights(nc):
    """The rust scheduler splits every InstMatmult into an explicit
    InstLdweights + InstMatmult(ldweights=False) pair. The PE's weight
    registers persist across matmuls, so a reload of the exact same
    stationary AP is pure overhead (~104ns each). Remove redundant
    loads, folding any waits/updates into the following instruction."""
    import concourse.mybir as mybir

    def key(ld):
        ap = ld.ins[0]
        return (ap.memref, ap.offset, str(ap.ap), str(ap.dtype),
                str(ld.perf_mode), ld.is_transpose,
                str(ld.tile_position), str(ld.tile_size))

    removed = 0
    for blk in nc.main_func.blocks:
        cur = None
        out = []
        pe_following = None  # where to fold a removed ld's sync
        for inst in blk.instructions:
            eng = getattr(inst, "engine", None)
            if eng != mybir.EngineType.PE:
                out.append(inst)
                continue
            if isinstance(inst, mybir.InstLdweights):
                k = key(inst)
                if k == cur:
                    si = inst.sync_info
                    if si is not None and (si.on_wait or si.on_update):
                        pe_following = si  # fold into next PE inst
                    removed += 1
                    continue
                cur = k
                out.append(inst)
            elif isinstance(inst, mybir.InstMatmult):
                if inst.ldweights is not False:
                    cur = None  # self-loading matmul clobbers weights
                if pe_following is not None:
                    si = inst.sync_info
                    if si is None:
                        inst.sync_info = pe_following
                    else:
                        si.on_wait = list(pe_following.on_wait) + list(si.on_wait)
                        si.on_update = list(pe_following.on_update) + list(si.on_update)
                    pe_following = None
                out.append(inst)
            else:
                # NoOps/semaphores/drains on PE do not touch the array
                out.append(inst)
        assert pe_following is None
        blk.instructions[:] = out
    return removed


def _legalize_waits(nc, max_waits=1):
    """This walrus build accepts at most one sync wait per instruction.
    Hoist extra waits onto preceding NoOps on the same engine (same-engine
    program order preserves the gating semantics)."""
    for blk in nc.main_func.blocks:
        out = []
        for inst in blk.instructions:
            si = inst.sync_info
            if si is not None and len(si.on_wait) > max_waits:
                waits = list(si.on_wait)
                si.on_wait = waits[-max_waits:]
                for w in waits[:-max_waits]:
                    nop = mybir.InstNoOp(
                        name=nc.get_next_instruction_name(), ins=[], outs=[]
                    )
                    nop.engine = inst.engine
                    nop.sync_info = mybir.SyncInfo(on_wait=[w], on_update=[])
                    nc.register_instruction(nop)
                    out.append(nop)
            out.append(inst)
        blk.instructions[:] = out


# ---------------------------------------------------------------------------

B, S, D = 2, 2048, 1024
H, HD = 16, 64
SCALE = 1.0 / HD
NCORES = 8
HPC = 4          # heads per core
DHC = HPC * HD   # 256 head-dims per core
P = 128
KC = D // P      # 8 contraction chunks for projections
NSUP = KC // 2   # 4 fp8 DoubleRow super-chunks (256-deep each)
SC = S // P      # 16 seq chunks of 128
QB = 512         # seq_q block for PV / O-proj
NQB = S // QB    # 4

F32 = mybir.dt.float32
BF16 = mybir.dt.bfloat16
FP8 = mybir.dt.float8e4
DR = mybir.MatmulPerfMode.DoubleRow

_COMPILED = None


def _build_nc():
    nc = bass.Bass("TRN2", target_bir_lowering=False, debug=False,
                   num_devices=NCORES)

    xT = nc.declare_dram_parameter("xT", [D, S], BF16, isOutput=False)
    x8 = nc.declare_dram_parameter("x8", [D, S], FP8, isOutput=False)
    wq8 = nc.declare_dram_parameter("wq8", [D, DHC], FP8, isOutput=False)
    wk8 = nc.declare_dram_parameter("wk8", [D, DHC], FP8, isOutput=False)
    wv = nc.declare_dram_parameter("wv", [D, DHC], BF16, isOutput=False)
    wo = nc.declare_dram_parameter("wo", [DHC, D], BF16, isOutput=False)
    bq = nc.declare_dram_parameter("bq", [2, P, 1], F32, isOutput=False)
    bk = nc.declare_dram_parameter("bk", [2, P, 1], F32, isOutput=False)
    bv = nc.declare_dram_parameter("bv", [P, DHC], F32, isOutput=False)
    tri = nc.declare_dram_parameter("tri", [P, P], BF16, isOutput=False)
    outT = nc.declare_dram_parameter("outT", [D, S], BF16, isOutput=True)

    with tile.TileContext(nc) as tc:
        dmaq = [nc.sync, nc.scalar, nc.gpsimd]
        dq = [0]

        def dma(out_ap, in_ap):
            eng = dmaq[dq[0] % len(dmaq)]
            dq[0] += 1
            return eng.dma_start(out_ap, in_ap)

        with (
            tc.tile_pool(name="persist", bufs=1) as pp,
            tc.tile_pool(name="stage", bufs=2) as stage,
            tc.tile_pool(name="epool", bufs=4) as epool,
            tc.tile_pool(name="small", bufs=4) as small,
        ):
            # ---------------- Phase A: load, project ----------------
            xTb = [pp.tile([P, S], BF16, tag=f"xtb{k}", name=f"xtb{k}") for k in range(KC)]
            # fp8 moving operand for Q/K proj: per 256-deep super-chunk,
            # two 128-row planes side by side: [128, (plane, seq)]
            x8b = [pp.tile([P, 2 * S], FP8, tag=f"x8b{c}", name=f"x8b{c}")
                   for c in range(NSUP)]
            # fp8 stationary for Q/K proj, packed [128, (k, out-col)]
            wq8b = pp.tile([P, KC * DHC], FP8, tag="wq8b", name="wq8b")
            wk8b = pp.tile([P, KC * DHC], FP8, tag="wk8b", name="wk8b")
            wvb = pp.tile([P, KC * DHC], BF16, tag="wvb", name="wvb")
            wob = pp.tile([P, 2 * D], BF16, tag="wob", name="wob")
            QT = [pp.tile([P, S], BF16, tag=f"qt{m}", name=f"qt{m}") for m in range(2)]
            KT = [pp.tile([P, S], BF16, tag=f"kt{m}", name=f"kt{m}") for m in range(2)]
            # V with a ones column per head: [h0(64) 1 | h1(64) 1 | ...]
            Vb = [pp.tile([P, HPC * 65], BF16, tag=f"vb{s}", name=f"vb{s}") for s in range(SC)]
            OT = [pp.tile([P, S], BF16, tag=f"ot{m}", name=f"ot{m}") for m in range(2)]
            trib = pp.tile([P, P], BF16, tag="trib")
            ones1 = pp.tile([1, 64], BF16, tag="ones1")
            bq_sb = pp.tile([P, 2], F32, tag="bq")
            bk_sb = pp.tile([P, 2], F32, tag="bk")
            bv_bc = pp.tile([P, DHC], F32, tag="bvbc")

            def k3(t, width=DHC):
                return t[:].rearrange("p (k c) -> p k c", k=KC)

            def x83(c):
                return x8b[c][:].rearrange("p (two n) -> p two n", two=2)

            # DMA: super-chunk-major so the first Q-proj matmuls' deps
            # (wq8 supers, x8 supers) land first; 3 queues round-robin
            for c in range(NSUP):
                dma(k3(wq8b)[:, 2 * c:2 * c + 2, :],
                    wq8[2 * c * P:(2 * c + 2) * P, :]
                    .rearrange("(two p) n -> p two n", p=P))
                src8 = (x8[2 * c * P:(2 * c + 2) * P, :]
                        .rearrange("(two p) n -> p two n", p=P))
                dma(x83(c)[:, 0:1, :], src8[:, 0:1, :])
                dma(x83(c)[:, 1:2, :], src8[:, 1:2, :])
                dma(k3(wk8b)[:, 2 * c:2 * c + 2, :],
                    wk8[2 * c * P:(2 * c + 2) * P, :]
                    .rearrange("(two p) n -> p two n", p=P))
            for k in range(KC):
                dma(xTb[k][:], xT[k * P:(k + 1) * P, :])
                dma(k3(wvb)[:, k, :], wv[k * P:(k + 1) * P, :])
            dma(trib[:], tri[:, :])
            nc.gpsimd.memset(ones1[:], 1.0)
            nc.sync.dma_start(bq_sb[:, 0:1], bq[0])
            nc.sync.dma_start(bq_sb[:, 1:2], bq[1])
            nc.sync.dma_start(bk_sb[:, 0:1], bk[0])
            nc.sync.dma_start(bk_sb[:, 1:2], bk[1])
            nc.scalar.dma_start(bv_bc[:], bv[:, :])
            dma(wob[:].rearrange("p (c d) -> p c d", c=2),
                wo[:, :].rearrange("(c p) d -> p c d", p=P))

            with tc.tile_pool(name="apsum", bufs=8, space="PSUM") as aps:
                # QT / KT: out [dh-chunk 128, seq]; fp8 DoubleRow over
                # 256-deep super-chunks, super-outer / nb-inner
                proj_order = [(wq8b, QT, bq_sb, 0), (wk8b, KT, bk_sb, 0),
                              (wq8b, QT, bq_sb, 1), (wk8b, KT, bk_sb, 1),
                              None]
                for item in proj_order:
                    if item is None:
                        # V: out [seq chunk, 256] bf16; lhsT = xT chunk
                        for s in range(SC):
                            ps = aps.tile([P, QB], F32, tag="proj",
                                          name=f"vproj{s}")
                            for k in range(KC):
                                nc.tensor.matmul(
                                    ps[:, 0:DHC],
                                    xTb[k][:, s * P:(s + 1) * P],
                                    k3(wvb)[:, k, :],
                                    start=(k == 0), stop=(k == KC - 1))
                            vout = Vb[s][:].rearrange("p (h x) -> p h x", h=HPC)[:, :, 0:64]
                            psr = ps[:, 0:DHC].rearrange("p (h x) -> p h x", h=HPC)
                            bvr = bv_bc[:].rearrange("p (h x) -> p h x", h=HPC)
                            nc.vector.tensor_add(vout, psr, bvr)
                            ones = Vb[s][:].rearrange("p (h x) -> p h x", h=HPC)[:, :, 64:65]
                            nc.gpsimd.memset(ones, 1.0)
                        continue
                    (w8b, dst, bias, m) = item
                    ps = [aps.tile([P, QB], F32, tag="proj", name=f"pj{m}{nb}")
                          for nb in range(NQB)]
                    for c in range(NSUP):
                        lhsT = (k3(w8b)[:, 2 * c:2 * c + 2, m * P:(m + 1) * P])
                        for nb in range(NQB):
                            nc.tensor.matmul(
                                ps[nb][:], lhsT,
                                x83(c)[:, :, nb * QB:(nb + 1) * QB],
                                start=(c == 0), stop=(c == NSUP - 1),
                                perf_mode=DR)
                    for nb in range(NQB):
                        nc.vector.tensor_scalar_add(
                            dst[m][:, nb * QB:(nb + 1) * QB],
                            ps[nb][:],
                            bias[:, m:m + 1],
                        )

            # ---------------- Phase B: attention per head ----------------
            # Each head is processed in two q-halves so O' fits in 2 PSUM
            # banks, freeing 6 banks for a triple-buffered [128,1024]
            # scores pipeline with 1024-wide exp reads (the PE clock-gate
            # releases to 2.4GHz only under sustained activity, so the PE
            # stream must never drain; ScalarE instruction count is the
            # phase floor).
            QH = S // 2  # 1024 q columns per half
            with (
                tc.tile_pool(name="scpsum", bufs=3, space="PSUM") as scp,
                tc.tile_pool(name="opsum", bufs=1, space="PSUM") as opp,
            ):
                pending_norm = [None]

                def flush_norm():
                    if pending_norm[0] is not None:
                        pending_norm[0]()
                        pending_norm[0] = None

                for h in range(HPC):
                 m, poff = h // 2, 64 * (h % 2)
                 kt_h = KT[m][poff:poff + 64, :]
                 qt_h = QT[m][poff:poff + 64, :]
                 o_sb = small.tile([65, S], F32, tag="osb", bufs=2)
                 for half in range(2):
                    q0 = half * QH
                    jc0 = 8 * half
                    ops = opp.tile([65, QH], F32, tag="oacc", name="oacc")

                    def pv_piece(jc, e, lqb, q0=q0, ops=ops, h=h):
                        # e holds cols [q0, q0+cw); global q-block 2*half+lqb
                        W = P * (jc + 1)
                        gqb = q0 // QB + lqb
                        cw = min(QB, W - gqb * QB)
                        nc.tensor.matmul(
                            ops[:, lqb * QB:lqb * QB + cw],
                            Vb[jc][:, 65 * h:65 * h + 65],
                            e[:, lqb * QB:lqb * QB + cw],
                            start=(jc == 4 * gqb), stop=(jc == SC - 1),
                            skip_group_check=True)

                    prev = None  # (jc, e, npieces) pending PV
                    for jc in range(jc0, SC):
                        W = P * (jc + 1)
                        cw = min(W - q0, QH)   # cols [q0, q0+cw)
                        e = epool.tile([P, QH], BF16, tag="e")
                        nsc = (cw + QB - 1) // QB
                        ps = scp.tile([P, QH], F32, tag="sc")
                        if prev:
                            for i in range(prev[2]):
                                pv_piece(prev[0], prev[1], i)
                        for i in range(nsc):
                            c0 = i * QB
                            ccw = min(QB, cw - c0)
                            nc.tensor.matmul(
                                ps[:, c0:c0 + ccw],
                                kt_h[:, jc * P:(jc + 1) * P],
                                qt_h[:, q0 + c0:q0 + c0 + ccw],
                                start=True, stop=True)
                        nc.scalar.activation(
                            e[:, 0:cw], ps[:, 0:cw],
                            mybir.ActivationFunctionType.Exp,
                            scale=SCALE,
                        )
                        # mask the diagonal 128-block (lives in this half
                        # only while jc < jc0+8)
                        if jc < jc0 + 8:
                            dc = W - P - q0
                            nc.gpsimd.tensor_mul(
                                e[:, dc:dc + P], e[:, dc:dc + P], trib[:])
                        prev = (jc, e, (min(W, q0 + QH) - q0 + QB - 1) // QB)
                    for i in range(prev[2]):
                        pv_piece(prev[0], prev[1], i)

                    # evict O' half to SBUF per qb-block
                    for lqb in range(2):
                        nc.vector.tensor_copy(
                            o_sb[:, q0 + lqb * QB:q0 + (lqb + 1) * QB],
                            ops[:, lqb * QB:(lqb + 1) * QB])

                 def norm(m=m, poff=poff, o_sb=o_sb):
                    # per-qb 1/denom = exp(-ln d) on ScalarE (5e-5 rel
                    # err), K=1 bf16 broadcast matmul, multiply on DVE
                    lrow = small.tile([1, S], F32, tag="lrow", bufs=2)
                    rrow = small.tile([1, S], BF16, tag="rrow", bufs=2)
                    for qb in range(NQB):
                        sl = slice(qb * QB, (qb + 1) * QB)
                        nc.scalar.activation(
                            lrow[:, sl], o_sb[64:65, sl],
                            mybir.ActivationFunctionType.Ln)
                        with nc.allow_low_precision(
                                reason="bf16 softmax denom broadcast"):
                            nc.scalar.activation(
                                rrow[:, sl], lrow[:, sl],
                                mybir.ActivationFunctionType.Exp,
                                scale=-1.0)
                        rbp = scp.tile([P, QH], F32, tag="sc")
                        nc.tensor.matmul(
                            rbp[0:64, 0:QB], ones1[:], rrow[:, sl],
                            start=True, stop=True,
                        )
                        nc.vector.tensor_mul(
                            OT[m][poff:poff + 64, sl],
                            o_sb[0:64, sl],
                            rbp[0:64, 0:QB],
                        )

                 flush_norm()
                 pending_norm[0] = norm
                flush_norm()

            # ---------------- Phase C: output projection ----------------
            with tc.tile_pool(name="cpsum", bufs=8, space="PSUM") as cps:
                for mo in range(D // P):
                    ot = stage.tile([P, S], BF16, tag="outstage")
                    ps = [cps.tile([P, QB], F32, tag="oproj", name=f"op{qb}")
                          for qb in range(NQB)]
                    for c in range(2):
                        lhsT = wob[:].rearrange("p (c d) -> p c d", c=2)[
                            :, c, mo * P:(mo + 1) * P]
                        for qb in range(NQB):
                            nc.tensor.matmul(
                                ps[qb][:], lhsT,
                                OT[c][:, qb * QB:(qb + 1) * QB],
                                start=(c == 0), stop=(c == 1))
                    for qb in range(NQB):
                        if qb % 2 == 0:
                            nc.vector.tensor_copy(
                                ot[:, qb * QB:(qb + 1) * QB], ps[qb][:])
                        else:
                            nc.scalar.copy(
                                ot[:, qb * QB:(qb + 1) * QB], ps[qb][:])
                    dma(outT[mo * P:(mo + 1) * P, :], ot[:])
    _dedup_ldweights(nc)
    _legalize_waits(nc)
    return nc


def _get_nc():
    global _COMPILED
    if _COMPILED is None:
        _COMPILED = _build_nc()
    return _COMPILED


def _make_in_maps(x, wq, bq, wk, bk, wv, bv, wo, bo):
    import ml_dtypes
    bf16 = ml_dtypes.bfloat16
    fp8 = ml_dtypes.float8_e4m3  # TRN fp8e4: max normal 240
    tri = np.tril(np.ones((P, P), dtype=bf16))
    in_maps = []
    for c in range(NCORES):
        b, g = c // 4, c % 4
        cols = slice(DHC * g, DHC * (g + 1))
        xt = np.ascontiguousarray(x[b].T)
        in_maps.append({
            "xT": xt.astype(bf16),
            "x8": xt.astype(fp8),
            "wq8": np.ascontiguousarray(wq[:, cols]).astype(fp8),
            "wk8": np.ascontiguousarray(wk[:, cols]).astype(fp8),
            "wv": np.ascontiguousarray(wv[:, cols]).astype(bf16),
            "wo": np.ascontiguousarray(wo[cols, :]).astype(bf16),
            "bq": np.ascontiguousarray(bq[cols]).reshape(2, P, 1),
            "bk": np.ascontiguousarray(bk[cols]).reshape(2, P, 1),
            "bv": np.ascontiguousarray(np.broadcast_to(bv[cols].reshape(1, DHC), (P, DHC))),
            "tri": tri,
        })
    return in_maps


def kernel(x, wq, bq, wk, bk, wv, bv, wo, bo, _trace=False, _trace_kwargs=None):
    x = np.asarray(x, dtype=np.float32)
    assert x.shape == (B, S, D), x.shape
    nc = _get_nc()
    in_maps = _make_in_maps(
        x, np.asarray(wq), np.asarray(bq), np.asarray(wk), np.asarray(bk),
        np.asarray(wv), np.asarray(bv), np.asarray(wo), np.asarray(bo))
    kw = {}
    if _trace:
        kw = dict(trace=True, **(_trace_kwargs or {}))
    res = run_bass_kernel_spmd(nc, in_maps, list(range(NCORES)), **kw)
    out = np.empty((B, S, D), dtype=np.float32)
    for b in range(B):
        acc = np.zeros((D, S), dtype=np.float32)
        for g in range(4):
            acc += np.asarray(res.results[4 * b + g]["outT"], dtype=np.float32)
        out[b] = acc.T + np.asarray(bo, dtype=np.float32)
    kernel.last_result = res
    return out
